# revision 1
# baseline (speedup 1.0000x reference)
"""GCN 2-layer kernel for Trainium2, 8 NeuronCores (edge-parallel, dst-sharded).

Math: standard PyG GCNConv with self-loops factorizes as
    out = dinv (.) (A01 @ (dinv (.) (x@W))) + dinv^2 (.) (x@W) + b
where A01 is the 0/1 adjacency (no self-loops) and dinv = 1/sqrt(deg).
So no per-edge norm is needed: aggregation is a plain segment-sum of
gathered, pre-scaled feature rows.

Device strategy per core (SPMD, core j owns dst nodes [6250j, 6250(j+1))):
  A) transform own x shard: h = x@W1, table1 = bf16(dinv*h), selfterm kept
  B) AllGather table1 -> full [N,64] bf16 table in local DRAM
  C) indirect-DMA gather src rows for own (dst-sorted) edges; aggregate via
     one-hot matmul accumulated in PSUM per 128-dst block
  D) layer-2 transform per block, AllGather table2, aggregate again,
     log_softmax, write own output shard.
"""

import sys
import types
import numpy as np

# ---------------------------------------------------------------- constants
N = 50000
E = 800000
CIN = 64
CHID = 64
COUT = 40
CORES = 8
SHARD = N // CORES          # 6250 real nodes per core
RT = (SHARD + 127) // 128   # 49 row tiles / blocks per core
SHARD_PAD = RT * 128        # 6272 padded rows per core
NBLK = RT                   # dst blocks of 128 nodes
GATHER_GROUP = 7            # blocks per indirect-DMA gather instruction

_BF16 = None  # ml_dtypes bfloat16, resolved lazily


def _bf16():
    global _BF16
    if _BF16 is None:
        import ml_dtypes
        _BF16 = ml_dtypes.bfloat16
    return _BF16


# ------------------------------------------------------------- environment
_ENV_READY = False


def _ensure_env():
    """Make concourse importable and install the NTFF profile hook shim."""
    global _ENV_READY
    if _ENV_READY:
        return
    for p in ("/opt/trn_rl_repo",):
        if p not in sys.path:
            sys.path.append(p)
    try:
        import antenv
        if "antenv.axon_hooks" not in sys.modules:
            hooks = types.ModuleType("antenv.axon_hooks")
            hooks._hook = None

            def set_axon_ntff_profile_hook(h):
                hooks._hook = h

            def get_axon_ntff_profile_hook():
                return hooks._hook

            hooks.set_axon_ntff_profile_hook = set_axon_ntff_profile_hook
            hooks.get_axon_ntff_profile_hook = get_axon_ntff_profile_hook
            sys.modules["antenv.axon_hooks"] = hooks
            antenv.axon_hooks = hooks
            try:
                from trn_agent_boot.trn_boot import _ntff_profile_via_ctypes
                h = _ntff_profile_via_ctypes("/opt/axon/libaxon_pjrt.so")
                if h is not None:
                    hooks.set_axon_ntff_profile_hook(h)
            except Exception:
                pass
        from concourse import bass_utils
        bass_utils.upload_artifacts = lambda tmpdir: "local://" + str(tmpdir)
    except Exception:
        pass
    _ENV_READY = True


# ---------------------------------------------------------------- host prep
def _host_prep(edge_index):
    """Index-only preprocessing: degree counts + per-core dst-sorted,
    block-padded edge slot tables.

    Returns dict with per-core arrays:
      gidx  [CORES, 128, NBLK*TB] int32  gather row in the padded table
      dloc  [CORES, 128, NBLK*TB] f32    local dst within block, -1 = pad
      deg   [CORES, 128, NBLK]    f32    degree (incl self loop) per node
    and TB (tiles per block, uniform).
    """
    src = np.asarray(edge_index[0], dtype=np.int64)
    dst = np.asarray(edge_index[1], dtype=np.int64)

    deg = np.bincount(dst, minlength=N).astype(np.float32) + 1.0

    core = dst // SHARD
    loc = dst - core * SHARD
    blk = loc >> 7
    dl = (loc & 127).astype(np.int32)
    key = (core * NBLK + blk).astype(np.int64)

    order = np.argsort(key, kind="stable")
    key_s = key[order]
    counts = np.bincount(key_s, minlength=CORES * NBLK)
    # per-block-index tile count: max over cores (SPMD-uniform compile-time list)
    tiles_cb = (counts.reshape(CORES, NBLK) + 127) // 128
    TB_list = tuple(int(max(1, t)) for t in tiles_cb.max(axis=0))
    coff = np.zeros(NBLK, dtype=np.int64)
    np.cumsum(TB_list[:-1], out=coff[1:])
    cols = int(coff[-1] + TB_list[-1])

    starts = np.zeros(CORES * NBLK, dtype=np.int64)
    np.cumsum(counts[:-1], out=starts[1:])
    rank = np.arange(E, dtype=np.int64) - starts[key_s]

    # remap src node id -> row in the concatenated padded table
    src_s = src[order]
    home = src_s // SHARD
    table_row = (home * SHARD_PAD + (src_s - home * SHARD)).astype(np.int32)

    gidx = np.zeros((CORES, 128, cols), dtype=np.int32)
    dloc = np.full((CORES, 128, cols), -1.0, dtype=np.float32)

    col = coff[key_s % NBLK] + (rank >> 7)
    row = rank & 127
    core_s = key_s // NBLK
    gidx[core_s, row, col] = table_row
    dloc[core_s, row, col] = dl[order].astype(np.float32)

    deg_sh = np.ones((CORES, 128, NBLK), dtype=np.float32)
    nid = np.arange(SHARD, dtype=np.int64)
    for j in range(CORES):
        d = deg[j * SHARD:(j + 1) * SHARD]
        deg_sh[j, nid & 127, nid >> 7] = d

    return {"gidx": gidx, "dloc": dloc, "deg": deg_sh, "TB_list": TB_list}


# ------------------------------------------------------------ bass program
def _build_program(TB_list):
    import concourse.bacc as bacc
    import concourse.mybir as mybir
    import concourse.tile as tile
    from concourse import bass

    fp32 = mybir.dt.float32
    bf16 = mybir.dt.bfloat16
    i32 = mybir.dt.int32
    AF = mybir.ActivationFunctionType
    ALU = mybir.AluOpType

    nc = bacc.Bacc("TRN2", target_bir_lowering=False, debug=False,
                   num_devices=CORES)

    coff = [0] * NBLK
    for b in range(1, NBLK):
        coff[b] = coff[b - 1] + TB_list[b - 1]
    cols = coff[-1] + TB_list[-1]

    # kernel I/O
    x_in = nc.dram_tensor("x_shard", [SHARD_PAD, CIN], fp32, kind="ExternalInput")
    w1_in = nc.dram_tensor("W1", [CIN, CHID], fp32, kind="ExternalInput")
    b1_in = nc.dram_tensor("b1f", [128, CHID], fp32, kind="ExternalInput")
    w2_in = nc.dram_tensor("W2", [CHID, COUT], fp32, kind="ExternalInput")
    b2_in = nc.dram_tensor("b2f", [128, COUT], fp32, kind="ExternalInput")
    deg_in = nc.dram_tensor("deg", [128, NBLK], fp32, kind="ExternalInput")
    gidx_in = nc.dram_tensor("gidx", [128, cols], i32, kind="ExternalInput")
    dloc_in = nc.dram_tensor("dloc", [128, cols], fp32, kind="ExternalInput")
    out_t = nc.dram_tensor("out", [SHARD_PAD, COUT], fp32, kind="ExternalOutput")

    # internal DRAM: shard tables + allgathered full tables
    tb1_sh = nc.dram_tensor("tb1_shard", [SHARD_PAD, CHID], bf16)
    tb1 = nc.dram_tensor("tb1", [CORES * SHARD_PAD, CHID], bf16,
                         addr_space="Shared")
    tb2_sh = nc.dram_tensor("tb2_shard", [SHARD_PAD, COUT], bf16)
    tb2 = nc.dram_tensor("tb2", [CORES * SHARD_PAD, COUT], bf16,
                         addr_space="Shared")

    # NEFF-embedded constants
    ident_f = nc.inline_tensor(np.eye(128, dtype=np.float32), "ident_f")
    ident_b = nc.inline_tensor(np.eye(128, dtype=_bf16()), "ident_b")
    iota_b = nc.inline_tensor(
        np.tile(np.arange(128, dtype=_bf16())[None, :], (128, 1)), "iota_b")

    rg = [list(range(CORES))]

    with tile.TileContext(nc) as tc:
        with (
            tc.tile_pool(name="persist", bufs=1) as pp,
            tc.tile_pool(name="stream", bufs=3) as sp,
            tc.tile_pool(name="msgs", bufs=24) as mp,
            tc.tile_pool(name="oh", bufs=16) as ohp,
            tc.tile_pool(name="post", bufs=3) as qp,
            tc.tile_pool(name="ptrans", bufs=3, space="PSUM") as pt,
            tc.tile_pool(name="pmm", bufs=3, space="PSUM") as pm,
            tc.tile_pool(name="pagg", bufs=2, space="PSUM") as pa,
        ):
            # ---- constants / persistent state ----
            identf = pp.tile([128, 128], fp32, tag="identf")
            nc.sync.dma_start(out=identf[:], in_=ident_f[:, :])
            identb = pp.tile([128, 128], bf16, tag="identb")
            nc.sync.dma_start(out=identb[:], in_=ident_b[:, :])
            iotab = pp.tile([128, 128], bf16, tag="iotab")
            nc.sync.dma_start(out=iotab[:], in_=iota_b[:, :])

            w1 = pp.tile([CIN, CHID], fp32, tag="w1")
            nc.sync.dma_start(out=w1[:], in_=w1_in[:, :])
            w2f = pp.tile([CHID, COUT], fp32, tag="w2f")
            nc.sync.dma_start(out=w2f[:], in_=w2_in[:, :])
            w2 = pp.tile([CHID, COUT], bf16, tag="w2")
            nc.vector.tensor_copy(out=w2[:], in_=w2f[:])
            b1f = pp.tile([128, CHID], fp32, tag="b1f")
            nc.sync.dma_start(out=b1f[:], in_=b1_in[:, :])
            b2f = pp.tile([128, COUT], fp32, tag="b2f")
            nc.sync.dma_start(out=b2f[:], in_=b2_in[:, :])

            degt = pp.tile([128, NBLK], fp32, tag="degt")
            nc.sync.dma_start(out=degt[:], in_=deg_in[:, :])
            dinv2 = pp.tile([128, NBLK], fp32, tag="dinv2")
            nc.vector.reciprocal(out=dinv2[:], in_=degt[:])
            dinv = pp.tile([128, NBLK], fp32, tag="dinv")
            nc.scalar.sqrt(out=dinv[:], in_=dinv2[:])

            gidx = pp.tile([128, cols], i32, tag="gidx")
            nc.sync.dma_start(out=gidx[:], in_=gidx_in[:, :])
            dloc = pp.tile([128, cols], fp32, tag="dloc")
            nc.sync.dma_start(out=dloc[:], in_=dloc_in[:, :])

            st1 = pp.tile([128, NBLK * CHID], fp32, tag="st1")
            st2 = pp.tile([128, NBLK * COUT], fp32, tag="st2")

            # ---- phase A: feature transform of own shard ----
            for r in range(RT):
                xt = sp.tile([128, CIN], fp32, tag="xt")
                nc.sync.dma_start(out=xt[:], in_=x_in[128 * r:128 * (r + 1), :])
                xT_p = pt.tile([CIN, 128], fp32, tag="tp")
                nc.tensor.transpose(out=xT_p[:], in_=xt[:], identity=identf[:])
                xT = sp.tile([CIN, 128], fp32, tag="xT")
                nc.scalar.copy(out=xT[:], in_=xT_p[:])
                h_p = pm.tile([128, CHID], fp32, tag="mm")
                nc.tensor.matmul(out=h_p[:], lhsT=xT[:], rhs=w1[:],
                                 start=True, stop=True)
                # table row: bf16(dinv * h)
                t1t = sp.tile([128, CHID], bf16, tag="t1t")
                nc.scalar.activation(out=t1t[:], in_=h_p[:], func=AF.Copy,
                                     scale=dinv[:, r:r + 1])
                nc.sync.dma_start(out=tb1_sh[128 * r:128 * (r + 1), :],
                                  in_=t1t[:])
                # selfterm: dinv^2 * h + b1
                nc.vector.tensor_scalar(
                    out=st1[:, CHID * r:CHID * (r + 1)], in0=h_p[:],
                    scalar1=dinv2[:, r:r + 1], scalar2=None, op0=ALU.mult)
                nc.vector.tensor_tensor(
                    out=st1[:, CHID * r:CHID * (r + 1)],
                    in0=st1[:, CHID * r:CHID * (r + 1)], in1=b1f[:],
                    op=ALU.add)

            # ---- AllGather layer-1 table ----
            nc.gpsimd.collective_compute(
                "AllGather", ALU.bypass, replica_groups=rg,
                ins=[tb1_sh.ap().opt()], outs=[tb1.ap().opt()])

            # ---- phase C: layer-1 aggregation + fused layer-2 transform ----
            for b in range(NBLK):
                agg = pa.tile([128, CHID], fp32, tag="agg")
                for t in range(TB_list[b]):
                    c = coff[b] + t
                    msg = mp.tile([128, CHID], bf16, tag="msg1")
                    nc.gpsimd.indirect_dma_start(
                        out=msg[:], out_offset=None, in_=tb1[:, :],
                        in_offset=bass.IndirectOffsetOnAxis(
                            ap=gidx[:, c:c + 1], axis=0))
                    oh = ohp.tile([128, 128], bf16, tag="oh1")
                    nc.vector.tensor_scalar(
                        out=oh[:], in0=iotab[:], scalar1=dloc[:, c:c + 1],
                        scalar2=None, op0=ALU.is_equal)
                    nc.tensor.matmul(out=agg[:], lhsT=oh[:], rhs=msg[:],
                                     start=(t == 0), stop=(t == TB_list[b] - 1))
                # out1 = relu(dinv*agg + st1)  (relu on DVE: max with 0)
                o1f = qp.tile([128, CHID], fp32, tag="o1f")
                nc.vector.tensor_scalar(out=o1f[:], in0=agg[:],
                                        scalar1=dinv[:, b:b + 1],
                                        scalar2=None, op0=ALU.mult)
                nc.vector.tensor_tensor(
                    out=o1f[:], in0=o1f[:],
                    in1=st1[:, CHID * b:CHID * (b + 1)], op=ALU.add)
                o1b = qp.tile([128, CHID], bf16, tag="o1b")
                nc.vector.tensor_scalar(out=o1b[:], in0=o1f[:], scalar1=0.0,
                                        scalar2=None, op0=ALU.max)
                # layer-2 transform of this block
                o1T_p = pt.tile([CHID, 128], bf16, tag="tp")
                nc.tensor.transpose(out=o1T_p[:], in_=o1b[:],
                                    identity=identb[:])
                o1T = qp.tile([CHID, 128], bf16, tag="o1T")
                nc.scalar.copy(out=o1T[:], in_=o1T_p[:])
                h2_p = pm.tile([128, COUT], fp32, tag="mm")
                nc.tensor.matmul(out=h2_p[:], lhsT=o1T[:], rhs=w2[:],
                                 start=True, stop=True)
                t2t = qp.tile([128, COUT], bf16, tag="t2t")
                nc.scalar.activation(out=t2t[:], in_=h2_p[:], func=AF.Copy,
                                     scale=dinv[:, b:b + 1])
                nc.sync.dma_start(out=tb2_sh[128 * b:128 * (b + 1), :],
                                  in_=t2t[:])
                nc.vector.tensor_scalar(
                    out=st2[:, COUT * b:COUT * (b + 1)], in0=h2_p[:],
                    scalar1=dinv2[:, b:b + 1], scalar2=None, op0=ALU.mult)
                nc.vector.tensor_tensor(
                    out=st2[:, COUT * b:COUT * (b + 1)],
                    in0=st2[:, COUT * b:COUT * (b + 1)], in1=b2f[:],
                    op=ALU.add)

            # ---- AllGather layer-2 table ----
            nc.gpsimd.collective_compute(
                "AllGather", ALU.bypass, replica_groups=rg,
                ins=[tb2_sh.ap().opt()], outs=[tb2.ap().opt()])

            # ---- phase E: layer-2 aggregation into O2, then batched softmax ----
            o2big = pp.tile([128, NBLK * COUT], fp32, tag="o2big")
            for b in range(NBLK):
                agg = pa.tile([128, COUT], fp32, tag="agg")
                for t in range(TB_list[b]):
                    c = coff[b] + t
                    msg = mp.tile([128, COUT], bf16, tag="msg2")
                    nc.gpsimd.indirect_dma_start(
                        out=msg[:], out_offset=None, in_=tb2[:, :],
                        in_offset=bass.IndirectOffsetOnAxis(
                            ap=gidx[:, c:c + 1], axis=0))
                    oh = ohp.tile([128, 128], bf16, tag="oh2")
                    nc.vector.tensor_scalar(
                        out=oh[:], in0=iotab[:], scalar1=dloc[:, c:c + 1],
                        scalar2=None, op0=ALU.is_equal)
                    nc.tensor.matmul(out=agg[:], lhsT=oh[:], rhs=msg[:],
                                     start=(t == 0), stop=(t == TB_list[b] - 1))
                nc.vector.tensor_scalar(
                    out=o2big[:, COUT * b:COUT * (b + 1)], in0=agg[:],
                    scalar1=dinv[:, b:b + 1], scalar2=None, op0=ALU.mult)
                nc.vector.tensor_tensor(
                    out=o2big[:, COUT * b:COUT * (b + 1)],
                    in0=o2big[:, COUT * b:COUT * (b + 1)],
                    in1=st2[:, COUT * b:COUT * (b + 1)], op=ALU.add)
            # batched log_softmax over all blocks: out = (o2-m) - ln(sum(exp(o2-m)))
            o2v = o2big[:].rearrange("p (b c) -> p b c", c=COUT)
            m = pp.tile([128, NBLK], fp32, tag="m")
            nc.vector.tensor_reduce(out=m[:], in_=o2v, axis=mybir.AxisListType.X,
                                    op=ALU.max)
            o2m = pp.tile([128, NBLK * COUT], fp32, tag="o2m")
            mb = m[:].to_broadcast([128, NBLK, COUT])
            nc.vector.tensor_tensor(
                out=o2m[:].rearrange("p (b c) -> p b c", c=COUT),
                in0=o2v, in1=mb, op=ALU.subtract)
            ex = pp.tile([128, NBLK * COUT], fp32, tag="exb")
            nc.scalar.activation(out=ex[:], in_=o2m[:], func=AF.Exp)
            s = pp.tile([128, NBLK], fp32, tag="s")
            nc.vector.tensor_reduce(out=s[:],
                                    in_=ex[:].rearrange("p (b c) -> p b c", c=COUT),
                                    axis=mybir.AxisListType.X, op=ALU.add)
            lns = pp.tile([128, NBLK], fp32, tag="lns")
            nc.scalar.activation(out=lns[:], in_=s[:], func=AF.Ln)
            of = pp.tile([128, NBLK * COUT], fp32, tag="of")
            lnb = lns[:].to_broadcast([128, NBLK, COUT])
            nc.vector.tensor_tensor(
                out=of[:].rearrange("p (b c) -> p b c", c=COUT),
                in0=o2m[:].rearrange("p (b c) -> p b c", c=COUT),
                in1=lnb, op=ALU.subtract)
            nc.sync.dma_start(
                out=out_t[:, :].rearrange("(b p) c -> p b c", p=128),
                in_=of[:].rearrange("p (b c) -> p b c", c=COUT))

    nc.compile()
    return nc


_PROGRAM_CACHE = {}


def _get_program(TB_list):
    if TB_list not in _PROGRAM_CACHE:
        _PROGRAM_CACHE[TB_list] = _build_program(TB_list)
    return _PROGRAM_CACHE[TB_list]


# ------------------------------------------------------------------ runner
def _run(inputs, trace=False, tmpdir=None):
    _ensure_env()
    from concourse.bass_utils import run_bass_kernel_spmd

    x = np.asarray(inputs["x"], dtype=np.float32)
    W1 = np.asarray(inputs["W1"], dtype=np.float32)
    b1 = np.asarray(inputs["b1"], dtype=np.float32)
    W2 = np.asarray(inputs["W2"], dtype=np.float32)
    b2 = np.asarray(inputs["b2"], dtype=np.float32)

    prep = _host_prep(np.asarray(inputs["edge_index"]))
    nc = _get_program(prep["TB_list"])

    b1f = np.tile(b1[None, :], (128, 1)).astype(np.float32)
    b2f = np.tile(b2[None, :], (128, 1)).astype(np.float32)

    in_maps = []
    for j in range(CORES):
        xs = np.zeros((SHARD_PAD, CIN), dtype=np.float32)
        xs[:SHARD] = x[j * SHARD:(j + 1) * SHARD]
        in_maps.append({
            "x_shard": xs,
            "W1": W1, "b1f": b1f, "W2": W2, "b2f": b2f,
            "deg": np.ascontiguousarray(prep["deg"][j]),
            "gidx": np.ascontiguousarray(prep["gidx"][j]),
            "dloc": np.ascontiguousarray(prep["dloc"][j]),
        })

    res = run_bass_kernel_spmd(nc, in_maps, core_ids=list(range(CORES)),
                               trace=trace, tmpdir=tmpdir,
                               trace_cores=[0] if trace else None)
    out = np.concatenate(
        [res.results[j]["out"][:SHARD] for j in range(CORES)], axis=0)
    return out.astype(np.float32), res


def kernel(**inputs) -> np.ndarray:
    out, _ = _run(inputs, trace=False)
    return out



# revision 4
# speedup vs baseline: 4.8406x; 4.8406x over previous
"""GCN 2-layer kernel for Trainium2, 8 NeuronCores (edge-parallel, dst-sharded).

Math: standard PyG GCNConv with self-loops factorizes as
    out = dinv (.) (A01 @ (dinv (.) (x@W))) + dinv^2 (.) (x@W) + b
where A01 is the 0/1 adjacency (no self-loops) and dinv = 1/sqrt(deg).
So no per-edge norm is needed: aggregation is a plain segment-sum of
gathered, pre-scaled feature rows.

Device strategy per core (SPMD, core j owns dst nodes [6250j, 6250(j+1))):
  A) transform own x shard: h = x@W1, table1 = bf16(dinv*h), selfterm kept
  B) AllGather table1 -> full [N,64] bf16 table in local DRAM
  C) indirect-DMA gather src rows for own (dst-sorted) edges; aggregate via
     one-hot matmul accumulated in PSUM per 128-dst block
  D) layer-2 transform per block, AllGather table2, aggregate again,
     log_softmax, write own output shard.
"""

import sys
import types
import numpy as np

# ---------------------------------------------------------------- constants
N = 50000
E = 800000
CIN = 64
CHID = 64
COUT = 40
CORES = 8
SHARD = N // CORES          # 6250 real nodes per core
RT = (SHARD + 127) // 128   # 49 row tiles / blocks per core
SHARD_PAD = RT * 128        # 6272 padded rows per core
NBLK = RT                   # dst blocks of 128 nodes
GATHER_GROUP = 7            # blocks per indirect-DMA gather instruction

_BF16 = None  # ml_dtypes bfloat16, resolved lazily


def _bf16():
    global _BF16
    if _BF16 is None:
        import ml_dtypes
        _BF16 = ml_dtypes.bfloat16
    return _BF16


# ------------------------------------------------------------- environment
_ENV_READY = False


def _ensure_env():
    """Make concourse importable and install the NTFF profile hook shim."""
    global _ENV_READY
    if _ENV_READY:
        return
    for p in ("/opt/trn_rl_repo",):
        if p not in sys.path:
            sys.path.append(p)
    try:
        import antenv
        if "antenv.axon_hooks" not in sys.modules:
            hooks = types.ModuleType("antenv.axon_hooks")
            hooks._hook = None

            def set_axon_ntff_profile_hook(h):
                hooks._hook = h

            def get_axon_ntff_profile_hook():
                return hooks._hook

            hooks.set_axon_ntff_profile_hook = set_axon_ntff_profile_hook
            hooks.get_axon_ntff_profile_hook = get_axon_ntff_profile_hook
            sys.modules["antenv.axon_hooks"] = hooks
            antenv.axon_hooks = hooks
            try:
                from trn_agent_boot.trn_boot import _ntff_profile_via_ctypes
                h = _ntff_profile_via_ctypes("/opt/axon/libaxon_pjrt.so")
                if h is not None:
                    hooks.set_axon_ntff_profile_hook(h)
            except Exception:
                pass
        from concourse import bass_utils
        bass_utils.upload_artifacts = lambda tmpdir: "local://" + str(tmpdir)
    except Exception:
        pass
    _ENV_READY = True


# ---------------------------------------------------------------- host prep
def _host_prep(edge_index):
    """Index-only preprocessing: degree counts + per-core dst-sorted,
    block-padded edge slot tables.

    Returns dict with per-core arrays:
      gidx  [CORES, 128, NBLK*TB] int32  gather row in the padded table
      dloc  [CORES, 128, NBLK*TB] f32    local dst within block, -1 = pad
      deg   [CORES, 128, NBLK]    f32    degree (incl self loop) per node
    and TB (tiles per block, uniform).
    """
    src = np.asarray(edge_index[0], dtype=np.int64)
    dst = np.asarray(edge_index[1], dtype=np.int64)

    deg = np.bincount(dst, minlength=N).astype(np.float32) + 1.0

    core = dst // SHARD
    loc = dst - core * SHARD
    blk = loc >> 7
    dl = (loc & 127).astype(np.int32)
    key = (core * NBLK + blk).astype(np.int64)

    order = np.argsort(key, kind="stable")
    key_s = key[order]
    counts = np.bincount(key_s, minlength=CORES * NBLK)
    # per-block-index tile count: max over cores (SPMD-uniform compile-time list)
    tiles_cb = (counts.reshape(CORES, NBLK) + 127) // 128
    TB_list = tuple(int(max(1, t)) for t in tiles_cb.max(axis=0))
    coff = np.zeros(NBLK, dtype=np.int64)
    np.cumsum(TB_list[:-1], out=coff[1:])
    cols = int(coff[-1] + TB_list[-1])

    starts = np.zeros(CORES * NBLK, dtype=np.int64)
    np.cumsum(counts[:-1], out=starts[1:])
    rank = np.arange(E, dtype=np.int64) - starts[key_s]

    # remap src node id -> row in the concatenated padded table
    src_s = src[order]
    home = src_s // SHARD
    table_row = (home * SHARD_PAD + (src_s - home * SHARD)).astype(np.int32)

    gidx = np.zeros((CORES, 128, cols), dtype=np.int32)
    dloc = np.full((CORES, 128, cols), -1.0, dtype=np.float32)

    col = coff[key_s % NBLK] + (rank >> 7)
    row = rank & 127
    core_s = key_s // NBLK
    gidx[core_s, row, col] = table_row
    dloc[core_s, row, col] = dl[order].astype(np.float32)

    deg_sh = np.ones((CORES, 128, NBLK), dtype=np.float32)
    nid = np.arange(SHARD, dtype=np.int64)
    for j in range(CORES):
        d = deg[j * SHARD:(j + 1) * SHARD]
        deg_sh[j, nid & 127, nid >> 7] = d

    return {"gidx": gidx, "dloc": dloc, "deg": deg_sh, "TB_list": TB_list}


# ------------------------------------------------------------ bass program
def _build_program(TB_list):
    import concourse.bacc as bacc
    import concourse.mybir as mybir
    import concourse.tile as tile
    from concourse import bass

    fp32 = mybir.dt.float32
    bf16 = mybir.dt.bfloat16
    i32 = mybir.dt.int32
    AF = mybir.ActivationFunctionType
    ALU = mybir.AluOpType

    nc = bacc.Bacc("TRN2", target_bir_lowering=False, debug=False,
                   num_devices=CORES)

    coff = [0] * NBLK
    for b in range(1, NBLK):
        coff[b] = coff[b - 1] + TB_list[b - 1]
    cols = coff[-1] + TB_list[-1]

    # kernel I/O
    x_in = nc.dram_tensor("x_shard", [SHARD_PAD, CIN], fp32, kind="ExternalInput")
    w1_in = nc.dram_tensor("W1", [CIN, CHID], fp32, kind="ExternalInput")
    b1_in = nc.dram_tensor("b1f", [128, CHID], fp32, kind="ExternalInput")
    w2_in = nc.dram_tensor("W2", [CHID, COUT], fp32, kind="ExternalInput")
    b2_in = nc.dram_tensor("b2f", [128, COUT], fp32, kind="ExternalInput")
    deg_in = nc.dram_tensor("deg", [128, NBLK], fp32, kind="ExternalInput")
    gidx_in = nc.dram_tensor("gidx", [128, cols], i32, kind="ExternalInput")
    dloc_in = nc.dram_tensor("dloc", [128, cols], fp32, kind="ExternalInput")
    out_t = nc.dram_tensor("out", [SHARD_PAD, COUT], fp32, kind="ExternalOutput")

    # internal DRAM: shard tables + allgathered full tables
    tb1_sh = nc.dram_tensor("tb1_shard", [SHARD_PAD, CHID], bf16)
    tb1 = nc.dram_tensor("tb1", [CORES * SHARD_PAD, CHID], bf16,
                         addr_space="Shared")
    tb2_sh = nc.dram_tensor("tb2_shard", [SHARD_PAD, COUT], bf16)
    tb2 = nc.dram_tensor("tb2", [CORES * SHARD_PAD, COUT], bf16,
                         addr_space="Shared")

    # NEFF-embedded constants
    ident_f = nc.inline_tensor(np.eye(128, dtype=np.float32), "ident_f")
    ident_b = nc.inline_tensor(np.eye(128, dtype=_bf16()), "ident_b")
    iota_b = nc.inline_tensor(
        np.tile(np.arange(128, dtype=_bf16())[None, :], (128, 1)), "iota_b")

    rg = [list(range(CORES))]

    with tile.TileContext(nc) as tc:
        with (
            tc.tile_pool(name="persist", bufs=1) as pp,
            tc.tile_pool(name="stream", bufs=3) as sp,
            tc.tile_pool(name="msgs", bufs=4) as mp,
            tc.tile_pool(name="oh", bufs=16) as ohp,
            tc.tile_pool(name="post", bufs=3) as qp,
            tc.tile_pool(name="ptrans", bufs=3, space="PSUM") as pt,
            tc.tile_pool(name="pmm", bufs=3, space="PSUM") as pm,
            tc.tile_pool(name="pagg", bufs=2, space="PSUM") as pa,
        ):
            # ---- constants / persistent state ----
            identf = pp.tile([128, 128], fp32, tag="identf")
            nc.sync.dma_start(out=identf[:], in_=ident_f[:, :])
            identb = pp.tile([128, 128], bf16, tag="identb")
            nc.sync.dma_start(out=identb[:], in_=ident_b[:, :])
            iotab = pp.tile([128, 128], bf16, tag="iotab")
            nc.sync.dma_start(out=iotab[:], in_=iota_b[:, :])

            w1 = pp.tile([CIN, CHID], fp32, tag="w1")
            nc.sync.dma_start(out=w1[:], in_=w1_in[:, :])
            w2f = pp.tile([CHID, COUT], fp32, tag="w2f")
            nc.sync.dma_start(out=w2f[:], in_=w2_in[:, :])
            w2 = pp.tile([CHID, COUT], bf16, tag="w2")
            nc.vector.tensor_copy(out=w2[:], in_=w2f[:])
            b1f = pp.tile([128, CHID], fp32, tag="b1f")
            nc.sync.dma_start(out=b1f[:], in_=b1_in[:, :])
            b2f = pp.tile([128, COUT], fp32, tag="b2f")
            nc.sync.dma_start(out=b2f[:], in_=b2_in[:, :])

            degt = pp.tile([128, NBLK], fp32, tag="degt")
            nc.sync.dma_start(out=degt[:], in_=deg_in[:, :])
            dinv2 = pp.tile([128, NBLK], fp32, tag="dinv2")
            nc.vector.reciprocal(out=dinv2[:], in_=degt[:])
            dinv = pp.tile([128, NBLK], fp32, tag="dinv")
            nc.scalar.sqrt(out=dinv[:], in_=dinv2[:])

            gidx = pp.tile([128, cols], i32, tag="gidx")
            nc.sync.dma_start(out=gidx[:], in_=gidx_in[:, :])
            dloc = pp.tile([128, cols], fp32, tag="dloc")
            nc.sync.dma_start(out=dloc[:], in_=dloc_in[:, :])

            st1 = pp.tile([128, NBLK * CHID], fp32, tag="st1")
            st2 = pp.tile([128, NBLK * COUT], fp32, tag="st2")

            # ---- phase A: feature transform of own shard ----
            for r in range(RT):
                xt = sp.tile([128, CIN], fp32, tag="xt")
                nc.sync.dma_start(out=xt[:], in_=x_in[128 * r:128 * (r + 1), :])
                xT_p = pt.tile([CIN, 128], fp32, tag="tp")
                nc.tensor.transpose(out=xT_p[:], in_=xt[:], identity=identf[:])
                xT = sp.tile([CIN, 128], fp32, tag="xT")
                nc.scalar.copy(out=xT[:], in_=xT_p[:])
                h_p = pm.tile([128, CHID], fp32, tag="mm")
                nc.tensor.matmul(out=h_p[:], lhsT=xT[:], rhs=w1[:],
                                 start=True, stop=True)
                # table row: bf16(dinv * h)
                t1t = sp.tile([128, CHID], bf16, tag="t1t")
                nc.scalar.activation(out=t1t[:], in_=h_p[:], func=AF.Copy,
                                     scale=dinv[:, r:r + 1])
                nc.sync.dma_start(out=tb1_sh[128 * r:128 * (r + 1), :],
                                  in_=t1t[:])
                # selfterm: dinv^2 * h + b1
                nc.vector.tensor_scalar(
                    out=st1[:, CHID * r:CHID * (r + 1)], in0=h_p[:],
                    scalar1=dinv2[:, r:r + 1], scalar2=None, op0=ALU.mult)
                nc.vector.tensor_tensor(
                    out=st1[:, CHID * r:CHID * (r + 1)],
                    in0=st1[:, CHID * r:CHID * (r + 1)], in1=b1f[:],
                    op=ALU.add)

            # ---- AllGather layer-1 table ----
            nc.gpsimd.collective_compute(
                "AllGather", ALU.bypass, replica_groups=rg,
                ins=[tb1_sh.ap().opt()], outs=[tb1.ap().opt()])

            TBMAX = max(TB_list)

            # ---- phase C: layer-1 aggregation + fused layer-2 transform ----
            for b in range(NBLK):
                TB = TB_list[b]
                agg = pa.tile([128, CHID], fp32, tag="agg")
                msg = mp.tile([128, TBMAX * CHID], bf16, tag="msg1")
                nc.gpsimd.indirect_dma_start(
                    out=msg[:, :TB * CHID], out_offset=None, in_=tb1[:, :],
                    in_offset=bass.IndirectOffsetOnAxis(
                        ap=gidx[:, coff[b]:coff[b] + TB], axis=0))
                for t in range(TB):
                    c = coff[b] + t
                    oh = ohp.tile([128, 128], bf16, tag="oh1")
                    nc.vector.tensor_scalar(
                        out=oh[:], in0=iotab[:], scalar1=dloc[:, c:c + 1],
                        scalar2=None, op0=ALU.is_equal)
                    nc.tensor.matmul(out=agg[:], lhsT=oh[:],
                                     rhs=msg[:, t * CHID:(t + 1) * CHID],
                                     start=(t == 0), stop=(t == TB - 1))
                # out1 = relu(dinv*agg + st1)  (relu on DVE: max with 0)
                o1f = qp.tile([128, CHID], fp32, tag="o1f")
                nc.vector.tensor_scalar(out=o1f[:], in0=agg[:],
                                        scalar1=dinv[:, b:b + 1],
                                        scalar2=None, op0=ALU.mult)
                nc.vector.tensor_tensor(
                    out=o1f[:], in0=o1f[:],
                    in1=st1[:, CHID * b:CHID * (b + 1)], op=ALU.add)
                o1b = qp.tile([128, CHID], bf16, tag="o1b")
                nc.vector.tensor_scalar(out=o1b[:], in0=o1f[:], scalar1=0.0,
                                        scalar2=None, op0=ALU.max)
                # layer-2 transform of this block
                o1T_p = pt.tile([CHID, 128], bf16, tag="tp")
                nc.tensor.transpose(out=o1T_p[:], in_=o1b[:],
                                    identity=identb[:])
                o1T = qp.tile([CHID, 128], bf16, tag="o1T")
                nc.scalar.copy(out=o1T[:], in_=o1T_p[:])
                h2_p = pm.tile([128, COUT], fp32, tag="mm")
                nc.tensor.matmul(out=h2_p[:], lhsT=o1T[:], rhs=w2[:],
                                 start=True, stop=True)
                t2t = qp.tile([128, COUT], bf16, tag="t2t")
                nc.scalar.activation(out=t2t[:], in_=h2_p[:], func=AF.Copy,
                                     scale=dinv[:, b:b + 1])
                nc.sync.dma_start(out=tb2_sh[128 * b:128 * (b + 1), :],
                                  in_=t2t[:])
                nc.vector.tensor_scalar(
                    out=st2[:, COUT * b:COUT * (b + 1)], in0=h2_p[:],
                    scalar1=dinv2[:, b:b + 1], scalar2=None, op0=ALU.mult)
                nc.vector.tensor_tensor(
                    out=st2[:, COUT * b:COUT * (b + 1)],
                    in0=st2[:, COUT * b:COUT * (b + 1)], in1=b2f[:],
                    op=ALU.add)

            # ---- AllGather layer-2 table ----
            nc.gpsimd.collective_compute(
                "AllGather", ALU.bypass, replica_groups=rg,
                ins=[tb2_sh.ap().opt()], outs=[tb2.ap().opt()])

            # ---- phase E: layer-2 aggregation into O2, then batched softmax ----
            o2big = pp.tile([128, NBLK * COUT], fp32, tag="o2big")
            for b in range(NBLK):
                TB = TB_list[b]
                agg = pa.tile([128, COUT], fp32, tag="agg")
                msg = mp.tile([128, TBMAX * COUT], bf16, tag="msg2")
                nc.gpsimd.indirect_dma_start(
                    out=msg[:, :TB * COUT], out_offset=None, in_=tb2[:, :],
                    in_offset=bass.IndirectOffsetOnAxis(
                        ap=gidx[:, coff[b]:coff[b] + TB], axis=0))
                for t in range(TB):
                    c = coff[b] + t
                    oh = ohp.tile([128, 128], bf16, tag="oh2")
                    nc.vector.tensor_scalar(
                        out=oh[:], in0=iotab[:], scalar1=dloc[:, c:c + 1],
                        scalar2=None, op0=ALU.is_equal)
                    nc.tensor.matmul(out=agg[:], lhsT=oh[:],
                                     rhs=msg[:, t * COUT:(t + 1) * COUT],
                                     start=(t == 0), stop=(t == TB - 1))
                nc.vector.tensor_scalar(
                    out=o2big[:, COUT * b:COUT * (b + 1)], in0=agg[:],
                    scalar1=dinv[:, b:b + 1], scalar2=None, op0=ALU.mult)
                nc.vector.tensor_tensor(
                    out=o2big[:, COUT * b:COUT * (b + 1)],
                    in0=o2big[:, COUT * b:COUT * (b + 1)],
                    in1=st2[:, COUT * b:COUT * (b + 1)], op=ALU.add)
            # batched log_softmax over all blocks: out = (o2-m) - ln(sum(exp(o2-m)))
            o2v = o2big[:].rearrange("p (b c) -> p b c", c=COUT)
            m = pp.tile([128, NBLK], fp32, tag="m")
            nc.vector.tensor_reduce(out=m[:], in_=o2v, axis=mybir.AxisListType.X,
                                    op=ALU.max)
            o2m = pp.tile([128, NBLK * COUT], fp32, tag="o2m")
            mb = m[:].to_broadcast([128, NBLK, COUT])
            nc.vector.tensor_tensor(
                out=o2m[:].rearrange("p (b c) -> p b c", c=COUT),
                in0=o2v, in1=mb, op=ALU.subtract)
            ex = pp.tile([128, NBLK * COUT], fp32, tag="exb")
            nc.scalar.activation(out=ex[:], in_=o2m[:], func=AF.Exp)
            s = pp.tile([128, NBLK], fp32, tag="s")
            nc.vector.tensor_reduce(out=s[:],
                                    in_=ex[:].rearrange("p (b c) -> p b c", c=COUT),
                                    axis=mybir.AxisListType.X, op=ALU.add)
            lns = pp.tile([128, NBLK], fp32, tag="lns")
            nc.scalar.activation(out=lns[:], in_=s[:], func=AF.Ln)
            of = pp.tile([128, NBLK * COUT], fp32, tag="of")
            lnb = lns[:].to_broadcast([128, NBLK, COUT])
            nc.vector.tensor_tensor(
                out=of[:].rearrange("p (b c) -> p b c", c=COUT),
                in0=o2m[:].rearrange("p (b c) -> p b c", c=COUT),
                in1=lnb, op=ALU.subtract)
            nc.sync.dma_start(
                out=out_t[:, :].rearrange("(b p) c -> p b c", p=128),
                in_=of[:].rearrange("p (b c) -> p b c", c=COUT))

    nc.compile()
    return nc


_PROGRAM_CACHE = {}


def _get_program(TB_list):
    if TB_list not in _PROGRAM_CACHE:
        _PROGRAM_CACHE[TB_list] = _build_program(TB_list)
    return _PROGRAM_CACHE[TB_list]


# ------------------------------------------------------------------ runner
def _run(inputs, trace=False, tmpdir=None):
    _ensure_env()
    from concourse.bass_utils import run_bass_kernel_spmd

    x = np.asarray(inputs["x"], dtype=np.float32)
    W1 = np.asarray(inputs["W1"], dtype=np.float32)
    b1 = np.asarray(inputs["b1"], dtype=np.float32)
    W2 = np.asarray(inputs["W2"], dtype=np.float32)
    b2 = np.asarray(inputs["b2"], dtype=np.float32)

    prep = _host_prep(np.asarray(inputs["edge_index"]))
    nc = _get_program(prep["TB_list"])

    b1f = np.tile(b1[None, :], (128, 1)).astype(np.float32)
    b2f = np.tile(b2[None, :], (128, 1)).astype(np.float32)

    in_maps = []
    for j in range(CORES):
        xs = np.zeros((SHARD_PAD, CIN), dtype=np.float32)
        xs[:SHARD] = x[j * SHARD:(j + 1) * SHARD]
        in_maps.append({
            "x_shard": xs,
            "W1": W1, "b1f": b1f, "W2": W2, "b2f": b2f,
            "deg": np.ascontiguousarray(prep["deg"][j]),
            "gidx": np.ascontiguousarray(prep["gidx"][j]),
            "dloc": np.ascontiguousarray(prep["dloc"][j]),
        })

    res = run_bass_kernel_spmd(nc, in_maps, core_ids=list(range(CORES)),
                               trace=trace, tmpdir=tmpdir,
                               trace_cores=[0] if trace else None)
    out = np.concatenate(
        [res.results[j]["out"][:SHARD] for j in range(CORES)], axis=0)
    return out.astype(np.float32), res


def kernel(**inputs) -> np.ndarray:
    out, _ = _run(inputs, trace=False)
    return out

